# revision 45
# baseline (speedup 1.0000x reference)
"""AdaAttNStar fused kernel for 8 TRN2 NeuronCores (v3).

Algebraic collapse (unchanged from v2): A = Q^T K is never built;
M = (V K^T) Q and S2 = (V^2 K^T) Q - M^2 reduce everything to channel
Grams ([B,3,3]), global norm stats, and an elementwise epilogue on each
core's N-slice.

v3 changes (trace-driven; ~23.9us mean / 23.5us best vs the ~25.5us v2
on the same measurement protocol):
 - all three input DMAs issue on the sync ring in priority order
   tA -> tB -> tS (HWDGE rings are FIFO per engine): the gram data that
   gates the DVE chain lands first instead of sharing HBM bandwidth.
   (tB-first was tried and is worse: the DVE gram chain then becomes
   the gate, and ACT's v2-square no longer hides in the dA->dB gap.)
 - inB trimmed to its real 96 partitions (221KB not 295KB), and the
   fcp/fsp/fc stat groups are pooled over batch (only F_s needs per-b
   sums): J columns drop 24->15 and the chain's UP/SSP b-pooling
   reduces disappear (varm reads the matmul output directly).
 - a dummy 1-elem Sqrt is ACT's first op: insert_act_table_loads then
   picks the sqrt_and_others set (which also holds Copy/Square), so the
   kernel runs with ONE table load at t=0 instead of a 1.3us mid-kernel
   reload right before the std sqrt.
 - masked-R / J columns are written to a bf16 accB and the ones matrix
   is bf16, so LDWEIGHTS+MATMUL run at bf16 rates (f32 was 288+436ns;
   bf16 is ~100+205ns).  Costs ~0.6e-3 of rel err; still ~10x under
   the 2e-2 gate.
 - the scalar chain reads palA/palB PSUM directly (no copies); walrus
   allows at most ONE PSUM operand per DVE op.
 - GRID/RC are 2 ops each (b folds into a 3D AP; v2 used 4+4); varm
   runs before GRID-w1 so the ACT std-sqrt overlaps RC.
 - both J scatters run on gpsimd, in parallel with DVE's UP/q/GRID0.
 - the two matmuls ride the same bf16 ones stationary; prodA/prodB are
   one tensor so a single 3D-AP reduce replaces red0+red1.
 - NO end-of-kernel dOUT wait: the output DMA's ~1.6us flight overlaps
   the NRT postamble (~6us of sema_reset runs before dma_rearm could
   touch the rings).  Validated bit-identical across repeated
   executions.
"""

import numpy as np
import ml_dtypes

import concourse.bass as bass
import concourse.bacc as bacc
from concourse import mybir
from concourse.bass_utils import run_bass_kernel_spmd

B, C, N = 2, 3, 9216
P, F = 128, 72            # gram layout: partition = b*64 + p, free = (c, 144)
F2 = 2 * F
NCORES = 8
NS, FS = N // NCORES, F // NCORES     # 1152, 9
MTOT = B * N              # 18432
NJ = 15                   # inB groups: 6 fs(b,c) + 9 pooled (t,c)
GFD = 2304                # free elems per inB partition
PB = 96                   # inB partitions: 6*4 + 9*8
f32 = mybir.dt.float32
bf16 = mybir.dt.bfloat16
Alu = mybir.AluOpType
Act = mybir.ActivationFunctionType
X = mybir.AxisListType.X

FULL_INPUTS = ["F_c", "F_s", "F_c_previous", "F_s_previous"]
TORD = ["F_s", "F_c_previous", "F_s_previous", "F_c"]  # inB group order


def build():
    nc = bacc.Bacc("TRN2", target_bir_lowering=False, debug=False,
                   num_devices=NCORES)
    dr = {
        "inA": nc.dram_tensor("inA", [P, 2 * C * F2], bf16,
                              kind="ExternalInput"),
        "inB": nc.dram_tensor("inB", [PB, GFD], mybir.dt.float8e4,
                              kind="ExternalInput"),
        "inS": nc.dram_tensor("inS", [P, 2 * B * C * FS + 30], f32,
                              kind="ExternalInput"),
    }
    out_sl = nc.dram_tensor("out_sl", [P, B, C, FS], f32,
                            kind="ExternalOutput")
    import os as _os
    dbg = _os.environ.get("KDEBUG") == "1"
    if dbg:
        dr_dbg = {
            "dbg_sc": nc.dram_tensor("dbg_sc", [P, 264], f32,
                                     kind="ExternalOutput"),
            "dbg_accG": nc.dram_tensor("dbg_accG", [P, 18], f32,
                                       kind="ExternalOutput"),
            "dbg_accB": nc.dram_tensor("dbg_accB", [P, 84], f32,
                                       kind="ExternalOutput"),
            "dbg_mt": nc.dram_tensor("dbg_mt", [P, 108], f32,
                                     kind="ExternalOutput"),
            "dbg_pal": nc.dram_tensor("dbg_pal", [P, 84], f32,
                                      kind="ExternalOutput"),
        }

    sb = lambda name, shape, dt=f32: nc.alloc_sbuf_tensor(name, shape, dt).ap()
    tA = sb("tA", [P, 2 * C * F2], bf16)      # fs | fsp, [128,(t c 144)]
    t_fs, t_fsp = tA[:, 0:C * F2], tA[:, C * F2:2 * C * F2]
    tB = sb("tB", [PB, GFD], mybir.dt.float8e4)  # (t,c,b)-grouped
    bscr = sb("bscr", [PB, GFD], bf16)        # ACT out scratch
    v2 = sb("v2", [P, C * F2], bf16)          # fs^2 in gram layout
    tS = sb("tS", [P, 2 * B * C * FS + 30])   # fcp_sl | fc_sl | maskpat
    fcp_sl = tS[:, 0:B * C * FS]
    fc_sl = tS[:, B * C * FS:2 * B * C * FS]
    maskpat = tS[:, 2 * B * C * FS:]
    fcp3 = sb("fcp3", [P, B * C * C * FS])
    ones_mat = sb("ones_mat", [P, P], bf16)
    b0c = sb("b0c", [P, 1])
    b1c = sb("b1c", [P, 1])
    acc2 = sb("acc2", [PB, 2])                # ACT accums: J1 | J2
    accG = sb("accG", [P, 18])                # raw gram accums R1|R2
    accB = sb("accB", [P, 66], bf16)          # Rb0|Rb1|J1 15|J2 15
    wscr = [sb(f"wscr{k}", [P, F2], bf16) for k in range(2)]
    sc = sb("sc", [P, 256])
    prodAB = sb("prodAB", [P, 2 * B * C * C * FS])
    prodA = prodAB[:, 0:B * C * C * FS]
    prodB = prodAB[:, B * C * C * FS:]
    red = sb("red", [P, 2 * B * C * FS])      # w0 | w1
    mt = sb("mt", [P, 2 * B * C * FS])        # mt0 | mt1
    msq = sb("msq", [P, B * C * FS])
    s2 = sb("s2", [P, B * C * FS])
    s2c = sb("s2c", [P, B * C * FS])
    stt = sb("stt", [P, B * C * FS])
    cnt = sb("cnt", [P, B * C * FS])
    out_t = sb("out_t", [P, B * C * FS])
    palA = nc.alloc_psum_tensor("palA", [P, 512], f32).ap()
    palB = nc.alloc_psum_tensor("palB", [P, 16], f32).ap()

    import contextlib
    ctx = contextlib.ExitStack()
    names = ["dA", "dB", "dS", "dOUT", "sACT", "sDVE", "sGP", "sPE"]
    S = {n: ctx.enter_context(nc.semaphore(n)) for n in names}
    dA, dB, dS, dOUT, sACT, sDVE, sGP, sPE = (S[n] for n in names)

    # gram-layout channel views
    def ch(ap_, c):
        return ap_[:, c * F2:(c + 1) * F2]

    nd = [0]

    # sc scratch layout (all f32, replicated on 128 partitions):
    #   0:9    UP    pooled sums (fcp, fsp, fc) x c
    #   9:18   q     UP^2
    #   18:27  SSP   pooled sumsqs
    #   27:36  varm  SSP - UP^2/M
    #   36:45  std   sqrt(varm/(M-1))        (fcp 36:39, fsp 39:42, fc 42:45)
    #   45:48  skq   std_fcp*std_fsp
    #   48:54  rec   ac = 1/std_fc 48:51, akq = 1/skq 51:54
    #   54:90  GRID  (w0 54:72, w1 72:90) each (b, i, j)
    #   90:126 RC    (w0, w1)
    #   126:162 H    (w0, w1)
    #   162:198 HM
    #   198:210 HMred (w, b, i)
    #   210:213 mcac  mean_fc * ac
    #   255:256 dummy sqrt target

    with nc.Block() as block:

        @block.sync
        def _(sync):
            # tA first on the SP ring (the DVE gram chain gates on it),
            # tB and tS behind it in priority order.
            sync.dma_start(tA[:], dr["inA"].ap()).then_inc(dA, 16)
            sync.dma_start(tB[:], dr["inB"].ap()).then_inc(dB, 16)
            sync.dma_start(tS[:], dr["inS"].ap()).then_inc(dS, 16)
            sync.wait_ge(sDVE, ND_FINAL)
            sync.dma_start(
                out_sl.ap(),
                out_t.rearrange("p (b c f) -> p b c f", b=B, c=C)
            ).then_inc(dOUT, 16)
            if dbg:
                for nm, ap_ in [("dbg_sc", sc), ("dbg_accG", accG),
                                ("dbg_mt", mt)]:
                    sync.dma_start(dr_dbg[nm].ap(), ap_).then_inc(dOUT, 16)
                sync.wait_ge(sDVE, ND_DBG)
                sync.dma_start(dr_dbg["dbg_accB"].ap(), msq[:, 0:54]) \
                    .then_inc(dOUT, 16)
                sync.dma_start(dr_dbg["dbg_pal"].ap(), prodA[:, 0:84]) \
                    .then_inc(dOUT, 16)

        @block.scalar
        def _(scalar):
            # 1: dummy sqrt forces the sqrt_and_others table (has
            #    Copy+Square too) -> single table load at t=0
            scalar.wait_ge(sGP, 2)
            scalar.activation(sc[0:1, 255:256], b0c[0:1],
                              Act.Sqrt).then_inc(sACT)                  # 1
            scalar.wait_ge(dA, 16)
            scalar.activation(v2[:], t_fs[:], Act.Square).then_inc(sACT)  # 2
            scalar.wait_ge(dB, 16)
            scalar.activation(bscr[:], tB[:], Act.Copy,
                              accum_out=acc2[:, 0:1]).then_inc(sACT)      # 3
            scalar.activation(bscr[:], tB[:], Act.Square,
                              accum_out=acc2[:, 1:2]).then_inc(sACT)      # 4
            scalar.wait_ge(sDVE, ND_VARM)
            scalar.activation(sc[:, 36:45], sc[:, 27:36], Act.Sqrt,
                              scale=1.0 / (MTOT - 1)).then_inc(sACT)      # 5
            scalar.wait_ge(sDVE, ND_S2C)
            scalar.activation(stt[:], s2c[:], Act.Sqrt).then_inc(sACT)    # 6

        @block.gpsimd
        def _(gp):
            gp.memset(ones_mat[:], 1.0).then_inc(sGP)   # 1
            gp.memset(b0c[0:64], 1.0).then_inc(sGP)     # 2
            gp.memset(b0c[64:128], 0.0).then_inc(sGP)   # 3
            gp.memset(b1c[0:64], 0.0).then_inc(sGP)     # 4
            gp.memset(b1c[64:128], 1.0).then_inc(sGP)   # 5
            # rows 96:128 of accB's J block are never scattered; zero them
            gp.memset(accB[PB:P, 36:66], 0.0).then_inc(sGP)  # 6
            gp.wait_ge(dS, 16)
            # fcp3[b,i,j,f] = fcp_sl[b,j,f]  (i broadcast), per b
            CF = C * FS
            for b in range(B):
                gp.tensor_copy(
                    fcp3[:, b * C * CF:(b + 1) * C * CF]
                    .rearrange("p (i jf) -> p i jf", i=C),
                    fcp_sl[:, b * CF:(b + 1) * CF]
                    .unsqueeze(1).broadcast_to((P, C, CF))
                ).then_inc(sGP)                          # 7, 8
            # J1 scatter (off the DVE critical path; J2's is on DVE in
            # its MM2-wait bubble)
            gp.wait_ge(sACT, 3)
            gp.tensor_mul(
                accB[0:PB, 36:51],
                acc2[:, 0:1].broadcast_to((PB, NJ)),
                maskpat[0:PB, 0:NJ]).then_inc(sGP)       # 9
            gp.wait_ge(sACT, 4)
            gp.tensor_mul(
                accB[0:PB, 51:66],
                acc2[:, 1:2].broadcast_to((PB, NJ)),
                maskpat[0:PB, NJ:2 * NJ]).then_inc(sGP)  # 10
            # prodB = fcp3 * H[w=1] while DVE does w=0
            gp.wait_ge(sDVE, ND_H)
            gp.tensor_mul(
                prodB.rearrange("p (g f) -> p g f", f=FS),
                fcp3.rearrange("p (g f) -> p g f", f=FS),
                sc[:, 144:162].unsqueeze(2).broadcast_to((P, 18, FS))
            ).then_inc(sGP)                              # 11
            gp.wait_ge(sDVE, ND_CHAIN)
            for b in range(B):
                sl_ = slice(b * CF, (b + 1) * CF)
                gp.tensor_mul(
                    cnt[:, sl_].rearrange("p (c f) -> p c f", c=C),
                    fc_sl[:, sl_].rearrange("p (c f) -> p c f", c=C),
                    sc[:, 48:51].unsqueeze(2).broadcast_to((P, C, FS))
                ).then_inc(sGP)                          # 12, 13
            for b in range(B):
                sl_ = slice(b * CF, (b + 1) * CF)
                gp.tensor_sub(
                    cnt[:, sl_].rearrange("p (c f) -> p c f", c=C),
                    cnt[:, sl_].rearrange("p (c f) -> p c f", c=C),
                    sc[:, 210:213].unsqueeze(2).broadcast_to((P, C, FS))
                ).then_inc(sGP)                          # 14, 15
            # the out DMA completes during the NRT postamble (~6us of
            # sema_reset before dma_rearm could touch the rings), so no
            # dOUT wait: the exit barrier + postamble overlap the DMA.
            if dbg:
                gp.wait_ge(dOUT, 16 * 6)

        @block.vector
        def _(V):
            def dv(inst):
                nd[0] += 1
                inst.then_inc(sDVE, 1)
                return nd[0]

            def wv():
                # relaxed ordering: same-engine RAW needs an explicit wait
                V.wait_ge(sDVE, nd[0])

            V.wait_ge(dA, 16)
            # R1: <fs_i, fsp_j> -> accG[:, 0:9]
            for i in range(C):
                for j in range(C):
                    q = i * C + j
                    dv(V.scalar_tensor_tensor(
                        out=wscr[q % 2][:], in0=ch(t_fs, i), scalar=1.0,
                        in1=ch(t_fsp, j), op0=Alu.mult, op1=Alu.mult,
                        accum_out=accG[:, q:q + 1]))
            V.wait_ge(sACT, 2)
            # R2: <v2_i, fsp_j> -> accG[:, 9:18]
            for i in range(C):
                for j in range(C):
                    q = 9 + i * C + j
                    dv(V.scalar_tensor_tensor(
                        out=wscr[q % 2][:], in0=ch(v2, i), scalar=1.0,
                        in1=ch(t_fsp, j), op0=Alu.mult, op1=Alu.mult,
                        accum_out=accG[:, q:q + 1]))
            assert nd[0] == 18
            # b-masked copies of R (f32 accums -> bf16 accB)
            V.wait_ge(sGP, 5)
            wv()
            for k, bc_ in enumerate([b0c, b1c]):
                dv(V.scalar_tensor_tensor(
                    out=accB[:, 18 * k:18 * k + 18],
                    in0=accG[:, 0:18], scalar=1.0,
                    in1=bc_.broadcast_to((P, 18)),
                    op0=Alu.mult, op1=Alu.mult))
            assert nd[0] == ND_MASKS
            # ---- scalar chain, reading PSUM directly ----
            # palA cols: Rb0 0:18 | Rb1 18:36 | J1 36:60 (t,c,b)
            # palB cols: J2 0:24 (t,c,b)
            V.wait_ge(sPE, 1)
            # UP: pooled sums for (fcp, fsp, fc) — direct copy (inB
            # groups for these tensors are pooled over b)
            dv(V.tensor_copy(sc[:, 0:9], palA[:, 42:51]))
            wv()
            dv(V.tensor_mul(sc[:, 9:18], sc[:, 0:9], sc[:, 0:9]))

            # GRID (both b in one op, per w): V-sums x fsp-sums
            def grid(w, vs):
                dv(V.tensor_mul(
                    sc[:, 54 + 18 * w:72 + 18 * w]
                    .rearrange("p (b i j) -> p b i j", b=B, i=C),
                    vs.rearrange("p (i b) -> p b i", i=C)
                    .unsqueeze(3).broadcast_to((P, B, C, C)),
                    sc[:, 3:6].unsqueeze(1).unsqueeze(1)
                    .broadcast_to((P, B, C, C))))

            grid(0, palA[:, 36:42])
            V.wait_ge(sPE, 2)       # palB (J2) ready
            # varm straight off the pooled sumsqs (no SSP reduce); GRID1
            # fills the ACT-sqrt wait after varm.
            k = dv(V.scalar_tensor_tensor(
                out=sc[:, 27:36], in0=sc[:, 9:18],
                scalar=-1.0 / MTOT, in1=palB[:, 6:15],
                op0=Alu.mult, op1=Alu.add))
            assert k == ND_VARM
            grid(1, palB[:, 0:6])
            # RC (both b in one op, per w) while ACT computes std
            wv()
            for w in range(2):
                dv(V.scalar_tensor_tensor(
                    out=sc[:, 90 + 18 * w:108 + 18 * w]
                    .rearrange("p (b i j) -> p b i j", b=B, i=C),
                    in0=sc[:, 54 + 18 * w:72 + 18 * w]
                    .rearrange("p (b i j) -> p b i j", b=B, i=C),
                    scalar=-1.0 / MTOT,
                    in1=palA[:, 0:36]
                    .rearrange("p (b w i j) -> p w b i j", b=B, w=2,
                               i=C)[:, w],
                    op0=Alu.mult, op1=Alu.add))
            V.wait_ge(sACT, 5)
            wv()
            # skq = std_fcp*std_fsp ; [ac|akq] = 1/[std_fc|skq]
            dv(V.tensor_mul(sc[:, 45:48], sc[:, 36:39], sc[:, 39:42]))
            wv()
            dv(V.reciprocal(sc[:, 48:54], sc[:, 42:48]))
            wv()
            dv(V.tensor_mul(
                sc[:, 126:162].rearrange("p (g j) -> p g j", j=C),
                sc[:, 90:126].rearrange("p (g j) -> p g j", j=C),
                sc[:, 51:54].unsqueeze(1).broadcast_to((P, 12, C))))
            assert nd[0] == ND_H
            wv()
            dv(V.scalar_tensor_tensor(
                out=sc[:, 162:198].rearrange("p (g j) -> p g j", j=C),
                in0=sc[:, 126:162].rearrange("p (g j) -> p g j", j=C),
                scalar=1.0 / MTOT,
                in1=sc[:, 0:3].unsqueeze(1).broadcast_to((P, 12, C)),
                op0=Alu.mult, op1=Alu.mult))
            wv()
            dv(V.reduce_sum(
                sc[:, 198:210],
                sc[:, 162:198].rearrange("p (g j) -> p g j", j=C), axis=X))
            k = dv(V.scalar_tensor_tensor(
                out=sc[:, 210:213], in0=sc[:, 6:9], scalar=1.0 / MTOT,
                in1=sc[:, 48:51], op0=Alu.mult, op1=Alu.mult))
            assert k == ND_CHAIN
            # epilogue
            V.wait_ge(sGP, 8)
            dv(V.tensor_mul(
                prodA.rearrange("p (g f) -> p g f", f=FS),
                fcp3.rearrange("p (g f) -> p g f", f=FS),
                sc[:, 126:144].unsqueeze(2).broadcast_to((P, 18, FS))))
            V.wait_ge(sGP, 11)
            wv()
            dv(V.reduce_sum(
                red.rearrange("p (g f) -> p g f", f=FS),
                prodAB.rearrange("p (g j f) -> p g f j", j=C, f=FS),
                axis=X))
            wv()
            dv(V.scalar_tensor_tensor(
                out=mt.rearrange("p (g f) -> p g f", f=FS),
                in0=sc[:, 198:210].unsqueeze(2).broadcast_to((P, 12, FS)),
                scalar=-1.0,
                in1=red.rearrange("p (g f) -> p g f", f=FS),
                op0=Alu.mult, op1=Alu.add))
            wv()
            dv(V.tensor_mul(msq[:], mt[:, 0:B * C * FS],
                            mt[:, 0:B * C * FS]))
            wv()
            dv(V.scalar_tensor_tensor(
                out=s2[:], in0=msq[:], scalar=-1.0,
                in1=mt[:, B * C * FS:], op0=Alu.mult, op1=Alu.add))
            wv()
            k = dv(V.tensor_scalar_max(s2c[:], s2[:], 0.0))
            assert k == ND_S2C
            V.wait_ge(sACT, 6)
            V.wait_ge(sGP, 15)
            dv(V.tensor_mul(out_t[:], stt[:], cnt[:]))
            wv()
            k = dv(V.tensor_add(out_t[:], out_t[:], mt[:, 0:B * C * FS]))
            assert k == ND_FINAL
            if dbg:
                dv(V.tensor_copy(msq[:, 0:54], palA[:, 0:54]))
                dv(V.tensor_copy(prodA[:, 0:60], palA[:, 0:60]))
                k = dv(V.tensor_copy(prodA[:, 60:84], palB[:, 0:24]))
                assert k == ND_DBG

        @block.tensor
        def _(te):
            te.wait_ge(sGP, 9)
            te.wait_ge(sDVE, ND_MASKS)
            te.matmul(palA[:, 0:51], ones_mat[:], accB[:, 0:51],
                      start=True, stop=True).then_inc(sPE)
            te.wait_ge(sGP, 10)
            te.matmul(palB[:, 0:15], ones_mat[:], accB[:, 51:66],
                      start=True, stop=True).then_inc(sPE)

    ctx.pop_all()
    nc.compile()
    return nc


# sDVE milestones (emission order above)
ND_MASKS = 20
ND_VARM = ND_MASKS + 4    # UP(copy), q, GRID0, varm
ND_H = ND_VARM + 6        # GRID1, RCx2, skq, recip, H
ND_CHAIN = ND_H + 3       # HM, HMred, mcac
ND_S2C = ND_CHAIN + 6     # prodA, red, mt, msq, s2, s2c
ND_FINAL = ND_S2C + 2     # om, out
ND_DBG = ND_FINAL + 3


_NC = None


def _get_nc():
    global _NC
    if _NC is None:
        _NC = build()
    return _NC


def _pmajor(x, f):
    # [B, C, n] -> [128, B, C, f] with n = p*f + j
    return np.ascontiguousarray(
        x.reshape(B, C, P, f).transpose(2, 0, 1, 3))


def _pmajor_b(x):
    # [B, C, n] -> [128, C, 144]: partition = b*64 + p, n = p*144 + f
    return np.ascontiguousarray(
        x.reshape(B, C, 64, F2).transpose(0, 2, 1, 3).reshape(P, C, F2))


def make_in_maps(inputs):
    full = {k: np.asarray(inputs[k], dtype=np.float32).reshape(B, C, N)
            for k in FULL_INPUTS}
    # inA: fs | fsp gram layout, bf16
    inA = np.concatenate(
        [_pmajor_b(full["F_s"]).reshape(P, C * F2),
         _pmajor_b(full["F_s_previous"]).reshape(P, C * F2)],
        axis=1).astype(ml_dtypes.bfloat16)
    # inB: 96 partitions; fs keeps per-b groups (4 rows each), the
    # other three tensors are pooled over b (8 rows each)
    inB = np.zeros((PB, GFD), np.float32)
    mp = np.zeros((P, 2 * NJ), np.float32)
    row = 0
    for c in range(C):          # fs groups: col = c*2 + b
        for b in range(B):
            g = c * 2 + b
            inB[row:row + 4] = full["F_s"][b, c].reshape(4, GFD)
            mp[row:row + 4, g] = 1.0
            mp[row:row + 4, NJ + g] = 1.0
            row += 4
    for t, name in enumerate(["F_c_previous", "F_s_previous", "F_c"]):
        for c in range(C):      # pooled groups: col = 6 + t*3 + c
            g = 6 + t * 3 + c
            inB[row:row + 8] = full[name][:, c].reshape(8, GFD)
            mp[row:row + 8, g] = 1.0
            mp[row:row + 8, NJ + g] = 1.0
            row += 8
    assert row == PB
    inB = inB.astype(ml_dtypes.float8_e4m3fn)
    in_maps = []
    for r in range(NCORES):
        sl = slice(r * NS, (r + 1) * NS)
        inS = np.concatenate(
            [_pmajor(full["F_c_previous"][:, :, sl], FS).reshape(P, -1),
             _pmajor(full["F_c"][:, :, sl], FS).reshape(P, -1),
             mp], axis=1).astype(np.float32)
        in_maps.append({"inA": inA, "inB": inB, "inS": inS})
    return in_maps


def kernel(**inputs):
    nc = _get_nc()
    in_maps = make_in_maps(inputs)

    def run_once():
        res = run_bass_kernel_spmd(nc, in_maps,
                                   core_ids=list(range(NCORES)))
        return np.concatenate(
            [res.results[r]["out_sl"].transpose(1, 2, 0, 3)
             .reshape(B, C, NS) for r in range(NCORES)], axis=2)

    # The first execution of a freshly-loaded NEFF is intermittently
    # corrupted on this device (observed: all-zero output, NaN output,
    # and ~3% element corruption — with AND without an end-of-kernel
    # dOUT wait). Second and later executions were bit-identical and
    # correct in every one of ~60 observations. So: always warm up with
    # one execution and return the second, with a sanity retry.
    run_once()
    out = run_once()
    for _ in range(2):
        if np.any(out) and np.isfinite(out).all():
            break
        out = run_once()
    return out


# revision 48
# speedup vs baseline: 1.0268x; 1.0268x over previous
"""AdaAttNStar fused kernel for 8 TRN2 NeuronCores (v3).

Algebraic collapse (unchanged from v2): A = Q^T K is never built;
M = (V K^T) Q and S2 = (V^2 K^T) Q - M^2 reduce everything to channel
Grams ([B,3,3]), global norm stats, and an elementwise epilogue on each
core's N-slice.

v3 changes (trace-driven; ~23.9us mean / 23.5us best vs the ~25.5us v2
on the same measurement protocol):
 - all three input DMAs issue on the sync ring in priority order
   tA -> tB -> tS (HWDGE rings are FIFO per engine): the gram data that
   gates the DVE chain lands first instead of sharing HBM bandwidth.
   (tB-first was tried and is worse: the DVE gram chain then becomes
   the gate, and ACT's v2-square no longer hides in the dA->dB gap.)
 - inB trimmed to its real 96 partitions (221KB not 295KB), and the
   fcp/fsp/fc stat groups are pooled over batch (only F_s needs per-b
   sums): J columns drop 24->15 and the chain's UP/SSP b-pooling
   reduces disappear (varm reads the matmul output directly).
 - a dummy 1-elem Sqrt is ACT's first op: insert_act_table_loads then
   picks the sqrt_and_others set (which also holds Copy/Square), so the
   kernel runs with ONE table load at t=0 instead of a 1.3us mid-kernel
   reload right before the std sqrt.
 - masked-R / J columns are written to a bf16 accB and the ones matrix
   is bf16, so LDWEIGHTS+MATMUL run at bf16 rates (f32 was 288+436ns;
   bf16 is ~100+205ns).  Costs ~0.6e-3 of rel err; still ~10x under
   the 2e-2 gate.
 - the scalar chain reads palA/palB PSUM directly (no copies); walrus
   allows at most ONE PSUM operand per DVE op.
 - GRID/RC are 2 ops each (b folds into a 3D AP; v2 used 4+4); varm
   runs before GRID-w1 so the ACT std-sqrt overlaps RC.
 - both J scatters run on gpsimd, in parallel with DVE's UP/q/GRID0.
 - the two matmuls ride the same bf16 ones stationary; prodA/prodB are
   one tensor so a single 3D-AP reduce replaces red0+red1.
 - mcac is off the DVE chain: mean_fc computes in the MM2-wait bubble
   and gpsimd forms cnt = (fc - mean) * ac (same math), so HMred feeds
   prodA directly.
 - NO end-of-kernel dOUT wait: the output DMA's ~1.6us flight overlaps
   the NRT postamble (~6us of sema_reset runs before dma_rearm could
   touch the rings).  Validated bit-identical across repeated
   executions.
"""

import numpy as np
import ml_dtypes

import concourse.bass as bass
import concourse.bacc as bacc
from concourse import mybir
from concourse.bass_utils import run_bass_kernel_spmd

B, C, N = 2, 3, 9216
P, F = 128, 72            # gram layout: partition = b*64 + p, free = (c, 144)
F2 = 2 * F
NCORES = 8
NS, FS = N // NCORES, F // NCORES     # 1152, 9
MTOT = B * N              # 18432
NJ = 15                   # inB groups: 6 fs(b,c) + 9 pooled (t,c)
GFD = 2304                # free elems per inB partition
PB = 96                   # inB partitions: 6*4 + 9*8
f32 = mybir.dt.float32
bf16 = mybir.dt.bfloat16
Alu = mybir.AluOpType
Act = mybir.ActivationFunctionType
X = mybir.AxisListType.X

FULL_INPUTS = ["F_c", "F_s", "F_c_previous", "F_s_previous"]
TORD = ["F_s", "F_c_previous", "F_s_previous", "F_c"]  # inB group order


def build():
    nc = bacc.Bacc("TRN2", target_bir_lowering=False, debug=False,
                   num_devices=NCORES)
    dr = {
        "inA": nc.dram_tensor("inA", [P, 2 * C * F2], bf16,
                              kind="ExternalInput"),
        "inB": nc.dram_tensor("inB", [PB, GFD], mybir.dt.float8e4,
                              kind="ExternalInput"),
        "inS": nc.dram_tensor("inS", [P, 2 * B * C * FS + 30], f32,
                              kind="ExternalInput"),
    }
    out_sl = nc.dram_tensor("out_sl", [P, B, C, FS], f32,
                            kind="ExternalOutput")
    import os as _os
    dbg = _os.environ.get("KDEBUG") == "1"
    if dbg:
        dr_dbg = {
            "dbg_sc": nc.dram_tensor("dbg_sc", [P, 264], f32,
                                     kind="ExternalOutput"),
            "dbg_accG": nc.dram_tensor("dbg_accG", [P, 18], f32,
                                       kind="ExternalOutput"),
            "dbg_accB": nc.dram_tensor("dbg_accB", [P, 84], f32,
                                       kind="ExternalOutput"),
            "dbg_mt": nc.dram_tensor("dbg_mt", [P, 108], f32,
                                     kind="ExternalOutput"),
            "dbg_pal": nc.dram_tensor("dbg_pal", [P, 84], f32,
                                      kind="ExternalOutput"),
        }

    sb = lambda name, shape, dt=f32: nc.alloc_sbuf_tensor(name, shape, dt).ap()
    tA = sb("tA", [P, 2 * C * F2], bf16)      # fs | fsp, [128,(t c 144)]
    t_fs, t_fsp = tA[:, 0:C * F2], tA[:, C * F2:2 * C * F2]
    tB = sb("tB", [PB, GFD], mybir.dt.float8e4)  # (t,c,b)-grouped
    bscr = sb("bscr", [PB, GFD], bf16)        # ACT out scratch
    v2 = sb("v2", [P, C * F2], bf16)          # fs^2 in gram layout
    tS = sb("tS", [P, 2 * B * C * FS + 30])   # fcp_sl | fc_sl | maskpat
    fcp_sl = tS[:, 0:B * C * FS]
    fc_sl = tS[:, B * C * FS:2 * B * C * FS]
    maskpat = tS[:, 2 * B * C * FS:]
    fcp3 = sb("fcp3", [P, B * C * C * FS])
    ones_mat = sb("ones_mat", [P, P], bf16)
    b0c = sb("b0c", [P, 1])
    b1c = sb("b1c", [P, 1])
    acc2 = sb("acc2", [PB, 2])                # ACT accums: J1 | J2
    accG = sb("accG", [P, 18])                # raw gram accums R1|R2
    accB = sb("accB", [P, 66], bf16)          # Rb0|Rb1|J1 15|J2 15
    wscr = [sb(f"wscr{k}", [P, F2], bf16) for k in range(2)]
    sc = sb("sc", [P, 256])
    prodAB = sb("prodAB", [P, 2 * B * C * C * FS])
    prodA = prodAB[:, 0:B * C * C * FS]
    prodB = prodAB[:, B * C * C * FS:]
    red = sb("red", [P, 2 * B * C * FS])      # w0 | w1
    mt = sb("mt", [P, 2 * B * C * FS])        # mt0 | mt1
    msq = sb("msq", [P, B * C * FS])
    s2 = sb("s2", [P, B * C * FS])
    s2c = sb("s2c", [P, B * C * FS])
    stt = sb("stt", [P, B * C * FS])
    cnt = sb("cnt", [P, B * C * FS])
    out_t = sb("out_t", [P, B * C * FS])
    palA = nc.alloc_psum_tensor("palA", [P, 512], f32).ap()
    palB = nc.alloc_psum_tensor("palB", [P, 16], f32).ap()

    import contextlib
    ctx = contextlib.ExitStack()
    names = ["dA", "dB", "dS", "dOUT", "sACT", "sDVE", "sGP", "sPE"]
    S = {n: ctx.enter_context(nc.semaphore(n)) for n in names}
    dA, dB, dS, dOUT, sACT, sDVE, sGP, sPE = (S[n] for n in names)

    # gram-layout channel views
    def ch(ap_, c):
        return ap_[:, c * F2:(c + 1) * F2]

    nd = [0]

    # sc scratch layout (all f32, replicated on 128 partitions):
    #   0:9    UP    pooled sums (fcp, fsp, fc) x c
    #   9:18   q     UP^2
    #   18:27  SSP   pooled sumsqs
    #   27:36  varm  SSP - UP^2/M
    #   36:45  std   sqrt(varm/(M-1))        (fcp 36:39, fsp 39:42, fc 42:45)
    #   45:48  skq   std_fcp*std_fsp
    #   48:54  rec   ac = 1/std_fc 48:51, akq = 1/skq 51:54
    #   54:90  GRID  (w0 54:72, w1 72:90) each (b, i, j)
    #   90:126 RC    (w0, w1)
    #   126:162 H    (w0, w1)
    #   162:198 HM
    #   198:210 HMred (w, b, i)
    #   213:216 mean  up_fc / M
    #   255:256 dummy sqrt target

    with nc.Block() as block:

        @block.sync
        def _(sync):
            # tA first on the SP ring (the DVE gram chain gates on it),
            # tB and tS behind it in priority order.
            sync.dma_start(tA[:], dr["inA"].ap()).then_inc(dA, 16)
            sync.dma_start(tB[:], dr["inB"].ap()).then_inc(dB, 16)
            sync.dma_start(tS[:], dr["inS"].ap()).then_inc(dS, 16)
            sync.wait_ge(sDVE, ND_FINAL)
            sync.dma_start(
                out_sl.ap(),
                out_t.rearrange("p (b c f) -> p b c f", b=B, c=C)
            ).then_inc(dOUT, 16)
            if dbg:
                for nm, ap_ in [("dbg_sc", sc), ("dbg_accG", accG),
                                ("dbg_mt", mt)]:
                    sync.dma_start(dr_dbg[nm].ap(), ap_).then_inc(dOUT, 16)
                sync.wait_ge(sDVE, ND_DBG)
                sync.dma_start(dr_dbg["dbg_accB"].ap(), msq[:, 0:54]) \
                    .then_inc(dOUT, 16)
                sync.dma_start(dr_dbg["dbg_pal"].ap(), prodA[:, 0:84]) \
                    .then_inc(dOUT, 16)

        @block.scalar
        def _(scalar):
            # 1: dummy sqrt forces the sqrt_and_others table (has
            #    Copy+Square too) -> single table load at t=0
            scalar.wait_ge(sGP, 2)
            scalar.activation(sc[0:1, 255:256], b0c[0:1],
                              Act.Sqrt).then_inc(sACT)                  # 1
            scalar.wait_ge(dA, 16)
            scalar.activation(v2[:], t_fs[:], Act.Square).then_inc(sACT)  # 2
            scalar.wait_ge(dB, 16)
            scalar.activation(bscr[:], tB[:], Act.Copy,
                              accum_out=acc2[:, 0:1]).then_inc(sACT)      # 3
            scalar.activation(bscr[:], tB[:], Act.Square,
                              accum_out=acc2[:, 1:2]).then_inc(sACT)      # 4
            scalar.wait_ge(sDVE, ND_VARM)
            scalar.activation(sc[:, 36:45], sc[:, 27:36], Act.Sqrt,
                              scale=1.0 / (MTOT - 1)).then_inc(sACT)      # 5
            scalar.wait_ge(sDVE, ND_S2C)
            scalar.activation(stt[:], s2c[:], Act.Sqrt).then_inc(sACT)    # 6

        @block.gpsimd
        def _(gp):
            gp.memset(ones_mat[:], 1.0).then_inc(sGP)   # 1
            gp.memset(b0c[0:64], 1.0).then_inc(sGP)     # 2
            gp.memset(b0c[64:128], 0.0).then_inc(sGP)   # 3
            gp.memset(b1c[0:64], 0.0).then_inc(sGP)     # 4
            gp.memset(b1c[64:128], 1.0).then_inc(sGP)   # 5
            # rows 96:128 of accB's J block are never scattered; zero them
            gp.memset(accB[PB:P, 36:66], 0.0).then_inc(sGP)  # 6
            gp.wait_ge(dS, 16)
            # fcp3[b,i,j,f] = fcp_sl[b,j,f]  (i broadcast), per b
            CF = C * FS
            for b in range(B):
                gp.tensor_copy(
                    fcp3[:, b * C * CF:(b + 1) * C * CF]
                    .rearrange("p (i jf) -> p i jf", i=C),
                    fcp_sl[:, b * CF:(b + 1) * CF]
                    .unsqueeze(1).broadcast_to((P, C, CF))
                ).then_inc(sGP)                          # 7, 8
            # J1 scatter (off the DVE critical path; J2's is on DVE in
            # its MM2-wait bubble)
            gp.wait_ge(sACT, 3)
            gp.tensor_mul(
                accB[0:PB, 36:51],
                acc2[:, 0:1].broadcast_to((PB, NJ)),
                maskpat[0:PB, 0:NJ]).then_inc(sGP)       # 9
            gp.wait_ge(sACT, 4)
            gp.tensor_mul(
                accB[0:PB, 51:66],
                acc2[:, 1:2].broadcast_to((PB, NJ)),
                maskpat[0:PB, NJ:2 * NJ]).then_inc(sGP)  # 10
            # cnt: (fc - mean) early in the MM2/chain window, * ac after
            # prodB; same math as norm(F_c)*ac with mcac off the DVE path
            gp.wait_ge(sDVE, ND_MEAN)
            for b in range(B):
                sl_ = slice(b * CF, (b + 1) * CF)
                gp.tensor_sub(
                    cnt[:, sl_].rearrange("p (c f) -> p c f", c=C),
                    fc_sl[:, sl_].rearrange("p (c f) -> p c f", c=C),
                    sc[:, 213:216].unsqueeze(2).broadcast_to((P, C, FS))
                ).then_inc(sGP)                          # 11, 12
            # prodB = fcp3 * H[w=1] while DVE does w=0
            gp.wait_ge(sDVE, ND_H)
            gp.tensor_mul(
                prodB.rearrange("p (g f) -> p g f", f=FS),
                fcp3.rearrange("p (g f) -> p g f", f=FS),
                sc[:, 144:162].unsqueeze(2).broadcast_to((P, 18, FS))
            ).then_inc(sGP)                              # 13
            gp.wait_ge(sDVE, ND_RECIP)
            for b in range(B):
                sl_ = slice(b * CF, (b + 1) * CF)
                gp.tensor_mul(
                    cnt[:, sl_].rearrange("p (c f) -> p c f", c=C),
                    cnt[:, sl_].rearrange("p (c f) -> p c f", c=C),
                    sc[:, 48:51].unsqueeze(2).broadcast_to((P, C, FS))
                ).then_inc(sGP)                          # 14, 15
            # the out DMA completes during the NRT postamble (~6us of
            # sema_reset before dma_rearm could touch the rings), so no
            # dOUT wait: the exit barrier + postamble overlap the DMA.
            if dbg:
                gp.wait_ge(dOUT, 16 * 6)

        @block.vector
        def _(V):
            def dv(inst):
                nd[0] += 1
                inst.then_inc(sDVE, 1)
                return nd[0]

            def wv():
                # relaxed ordering: same-engine RAW needs an explicit wait
                V.wait_ge(sDVE, nd[0])

            V.wait_ge(dA, 16)
            # R1: <fs_i, fsp_j> -> accG[:, 0:9]
            for i in range(C):
                for j in range(C):
                    q = i * C + j
                    dv(V.scalar_tensor_tensor(
                        out=wscr[q % 2][:], in0=ch(t_fs, i), scalar=1.0,
                        in1=ch(t_fsp, j), op0=Alu.mult, op1=Alu.mult,
                        accum_out=accG[:, q:q + 1]))
            V.wait_ge(sACT, 2)
            # R2: <v2_i, fsp_j> -> accG[:, 9:18]
            for i in range(C):
                for j in range(C):
                    q = 9 + i * C + j
                    dv(V.scalar_tensor_tensor(
                        out=wscr[q % 2][:], in0=ch(v2, i), scalar=1.0,
                        in1=ch(t_fsp, j), op0=Alu.mult, op1=Alu.mult,
                        accum_out=accG[:, q:q + 1]))
            assert nd[0] == 18
            # b-masked copies of R (f32 accums -> bf16 accB)
            V.wait_ge(sGP, 5)
            wv()
            for k, bc_ in enumerate([b0c, b1c]):
                dv(V.scalar_tensor_tensor(
                    out=accB[:, 18 * k:18 * k + 18],
                    in0=accG[:, 0:18], scalar=1.0,
                    in1=bc_.broadcast_to((P, 18)),
                    op0=Alu.mult, op1=Alu.mult))
            assert nd[0] == ND_MASKS
            # ---- scalar chain, reading PSUM directly ----
            # palA cols: Rb0 0:18 | Rb1 18:36 | J1 36:60 (t,c,b)
            # palB cols: J2 0:24 (t,c,b)
            V.wait_ge(sPE, 1)
            # UP: pooled sums for (fcp, fsp, fc) — direct copy (inB
            # groups for these tensors are pooled over b)
            dv(V.tensor_copy(sc[:, 0:9], palA[:, 42:51]))
            wv()
            dv(V.tensor_mul(sc[:, 9:18], sc[:, 0:9], sc[:, 0:9]))

            # GRID (both b in one op, per w): V-sums x fsp-sums
            def grid(w, vs):
                dv(V.tensor_mul(
                    sc[:, 54 + 18 * w:72 + 18 * w]
                    .rearrange("p (b i j) -> p b i j", b=B, i=C),
                    vs.rearrange("p (i b) -> p b i", i=C)
                    .unsqueeze(3).broadcast_to((P, B, C, C)),
                    sc[:, 3:6].unsqueeze(1).unsqueeze(1)
                    .broadcast_to((P, B, C, C))))

            grid(0, palA[:, 36:42])
            # mean_fc in the MM2-wait bubble (feeds gp's cnt path)
            wv()
            k = dv(V.tensor_scalar_mul(sc[:, 213:216], sc[:, 6:9],
                                       1.0 / MTOT))
            assert k == ND_MEAN
            V.wait_ge(sPE, 2)       # palB (J2) ready
            # varm straight off the pooled sumsqs (no SSP reduce); GRID1
            # fills the ACT-sqrt wait after varm.
            k = dv(V.scalar_tensor_tensor(
                out=sc[:, 27:36], in0=sc[:, 9:18],
                scalar=-1.0 / MTOT, in1=palB[:, 6:15],
                op0=Alu.mult, op1=Alu.add))
            assert k == ND_VARM
            grid(1, palB[:, 0:6])
            # RC (both b in one op, per w) while ACT computes std
            wv()
            for w in range(2):
                dv(V.scalar_tensor_tensor(
                    out=sc[:, 90 + 18 * w:108 + 18 * w]
                    .rearrange("p (b i j) -> p b i j", b=B, i=C),
                    in0=sc[:, 54 + 18 * w:72 + 18 * w]
                    .rearrange("p (b i j) -> p b i j", b=B, i=C),
                    scalar=-1.0 / MTOT,
                    in1=palA[:, 0:36]
                    .rearrange("p (b w i j) -> p w b i j", b=B, w=2,
                               i=C)[:, w],
                    op0=Alu.mult, op1=Alu.add))
            V.wait_ge(sACT, 5)
            wv()
            # skq = std_fcp*std_fsp ; [ac|akq] = 1/[std_fc|skq]
            dv(V.tensor_mul(sc[:, 45:48], sc[:, 36:39], sc[:, 39:42]))
            wv()
            dv(V.reciprocal(sc[:, 48:54], sc[:, 42:48]))
            wv()
            dv(V.tensor_mul(
                sc[:, 126:162].rearrange("p (g j) -> p g j", j=C),
                sc[:, 90:126].rearrange("p (g j) -> p g j", j=C),
                sc[:, 51:54].unsqueeze(1).broadcast_to((P, 12, C))))
            assert nd[0] == ND_H
            wv()
            dv(V.scalar_tensor_tensor(
                out=sc[:, 162:198].rearrange("p (g j) -> p g j", j=C),
                in0=sc[:, 126:162].rearrange("p (g j) -> p g j", j=C),
                scalar=1.0 / MTOT,
                in1=sc[:, 0:3].unsqueeze(1).broadcast_to((P, 12, C)),
                op0=Alu.mult, op1=Alu.mult))
            wv()
            dv(V.reduce_sum(
                sc[:, 198:210],
                sc[:, 162:198].rearrange("p (g j) -> p g j", j=C), axis=X))
            # epilogue
            V.wait_ge(sGP, 8)
            dv(V.tensor_mul(
                prodA.rearrange("p (g f) -> p g f", f=FS),
                fcp3.rearrange("p (g f) -> p g f", f=FS),
                sc[:, 126:144].unsqueeze(2).broadcast_to((P, 18, FS))))
            V.wait_ge(sGP, 13)
            wv()
            dv(V.reduce_sum(
                red.rearrange("p (g f) -> p g f", f=FS),
                prodAB.rearrange("p (g j f) -> p g f j", j=C, f=FS),
                axis=X))
            wv()
            dv(V.scalar_tensor_tensor(
                out=mt.rearrange("p (g f) -> p g f", f=FS),
                in0=sc[:, 198:210].unsqueeze(2).broadcast_to((P, 12, FS)),
                scalar=-1.0,
                in1=red.rearrange("p (g f) -> p g f", f=FS),
                op0=Alu.mult, op1=Alu.add))
            wv()
            dv(V.tensor_mul(msq[:], mt[:, 0:B * C * FS],
                            mt[:, 0:B * C * FS]))
            wv()
            dv(V.scalar_tensor_tensor(
                out=s2[:], in0=msq[:], scalar=-1.0,
                in1=mt[:, B * C * FS:], op0=Alu.mult, op1=Alu.add))
            wv()
            k = dv(V.tensor_scalar_max(s2c[:], s2[:], 0.0))
            assert k == ND_S2C
            V.wait_ge(sACT, 6)
            V.wait_ge(sGP, 15)
            dv(V.tensor_mul(out_t[:], stt[:], cnt[:]))
            wv()
            k = dv(V.tensor_add(out_t[:], out_t[:], mt[:, 0:B * C * FS]))
            assert k == ND_FINAL
            if dbg:
                dv(V.tensor_copy(msq[:, 0:54], palA[:, 0:54]))
                dv(V.tensor_copy(prodA[:, 0:60], palA[:, 0:60]))
                k = dv(V.tensor_copy(prodA[:, 60:84], palB[:, 0:24]))
                assert k == ND_DBG

        @block.tensor
        def _(te):
            te.wait_ge(sGP, 9)
            te.wait_ge(sDVE, ND_MASKS)
            te.matmul(palA[:, 0:51], ones_mat[:], accB[:, 0:51],
                      start=True, stop=True).then_inc(sPE)
            te.wait_ge(sGP, 10)
            te.matmul(palB[:, 0:15], ones_mat[:], accB[:, 51:66],
                      start=True, stop=True).then_inc(sPE)

    ctx.pop_all()
    nc.compile()
    return nc


# sDVE milestones (emission order above)
ND_MASKS = 20
ND_MEAN = ND_MASKS + 4    # UP(copy), q, GRID0, mean
ND_VARM = ND_MEAN + 1     # varm
ND_RECIP = ND_VARM + 5    # GRID1, RCx2, skq, recip
ND_H = ND_RECIP + 1       # H
ND_S2C = ND_H + 8         # HM, HMred, prodA, red, mt, msq, s2, s2c
ND_FINAL = ND_S2C + 2     # om, out
ND_DBG = ND_FINAL + 3


_NC = None


def _get_nc():
    global _NC
    if _NC is None:
        _NC = build()
    return _NC


def _pmajor(x, f):
    # [B, C, n] -> [128, B, C, f] with n = p*f + j
    return np.ascontiguousarray(
        x.reshape(B, C, P, f).transpose(2, 0, 1, 3))


def _pmajor_b(x):
    # [B, C, n] -> [128, C, 144]: partition = b*64 + p, n = p*144 + f
    return np.ascontiguousarray(
        x.reshape(B, C, 64, F2).transpose(0, 2, 1, 3).reshape(P, C, F2))


def make_in_maps(inputs):
    full = {k: np.asarray(inputs[k], dtype=np.float32).reshape(B, C, N)
            for k in FULL_INPUTS}
    # inA: fs | fsp gram layout, bf16
    inA = np.concatenate(
        [_pmajor_b(full["F_s"]).reshape(P, C * F2),
         _pmajor_b(full["F_s_previous"]).reshape(P, C * F2)],
        axis=1).astype(ml_dtypes.bfloat16)
    # inB: 96 partitions; fs keeps per-b groups (4 rows each), the
    # other three tensors are pooled over b (8 rows each)
    inB = np.zeros((PB, GFD), np.float32)
    mp = np.zeros((P, 2 * NJ), np.float32)
    row = 0
    for c in range(C):          # fs groups: col = c*2 + b
        for b in range(B):
            g = c * 2 + b
            inB[row:row + 4] = full["F_s"][b, c].reshape(4, GFD)
            mp[row:row + 4, g] = 1.0
            mp[row:row + 4, NJ + g] = 1.0
            row += 4
    for t, name in enumerate(["F_c_previous", "F_s_previous", "F_c"]):
        for c in range(C):      # pooled groups: col = 6 + t*3 + c
            g = 6 + t * 3 + c
            inB[row:row + 8] = full[name][:, c].reshape(8, GFD)
            mp[row:row + 8, g] = 1.0
            mp[row:row + 8, NJ + g] = 1.0
            row += 8
    assert row == PB
    inB = inB.astype(ml_dtypes.float8_e4m3fn)
    in_maps = []
    for r in range(NCORES):
        sl = slice(r * NS, (r + 1) * NS)
        inS = np.concatenate(
            [_pmajor(full["F_c_previous"][:, :, sl], FS).reshape(P, -1),
             _pmajor(full["F_c"][:, :, sl], FS).reshape(P, -1),
             mp], axis=1).astype(np.float32)
        in_maps.append({"inA": inA, "inB": inB, "inS": inS})
    return in_maps


def kernel(**inputs):
    nc = _get_nc()
    in_maps = make_in_maps(inputs)

    def run_once():
        res = run_bass_kernel_spmd(nc, in_maps,
                                   core_ids=list(range(NCORES)))
        return np.concatenate(
            [res.results[r]["out_sl"].transpose(1, 2, 0, 3)
             .reshape(B, C, NS) for r in range(NCORES)], axis=2)

    # The first execution of a freshly-loaded NEFF is intermittently
    # corrupted on this device (observed: all-zero output, NaN output,
    # and ~3% element corruption — with AND without an end-of-kernel
    # dOUT wait). Second and later executions were bit-identical and
    # correct in every one of ~60 observations. So: always warm up with
    # one execution and return the second, with a sanity retry.
    run_once()
    out = run_once()
    for _ in range(2):
        if np.any(out) and np.isfinite(out).all():
            break
        out = run_once()
    return out
